# revision 27
# baseline (speedup 1.0000x reference)
"""TRN2 Bass kernel for soft 2D polygon rasterization (1024x1024, 64-edge polygon).

Strategy (one SPMD program on 8 cores, per-core behavior fully data-driven):
  - Layout: y (rows) on partitions, x (columns) on the free axis; 64 tiles of
    [128 rows x 128 cols]; each core processes 8, assigned by a host-side
    load-balancing local search minimizing padded per-phase slot costs.
  - Inside/outside parity for ALL 8 tiles comes from ONE pair of fp16 matmuls:
    parT[i, k*128+x] = sum_y hist_k[y, x] * U[y <= i] with the shared
    triangular U as the stationary operand. The histogram carries the
    reference's exact f32 crossing parities plus +-1024 offsets implementing
    the bbox+threshold band mask in both axes and the base parity of each
    128-row band (all values fp16-exact).
  - Distance: only pixels within ~5.5 px of the boundary need true distance;
    the host culls per (row, tile) and PACKS a different edge into each
    partition lane of a slot, so a tile's slot count is the max per-row count.
    Each slot's ops are sliced to the slot's x-window (union of its edges'
    x-reach). Per edge slot:
      * one tensor_scalar computes w~ = s~*x + b~ (along-edge coordinate,
        scaled 2*sqrt(BIG)/L so the endpoint-slab test is w~^2 > BIG),
      * one fused custom DVE op (registered into concourse's custom-DVE
        table): d2 = min(d2, max(w~^2 - BIG, u^2)), where u (perpendicular
        distance) is generated internally by an ADD-scan. fp16 overflow of
        w~ saturates to +inf, which max/min handle correctly.
    Vertices (segment endpoints) cover the beyond-slab region exactly with
    one fused custom op over a ~16-col window: d2 = min(d2, (x-Ax)^2 + q).
  - Finals: one STT sd2 = (parT - 0.5)*d2 over all 4 edge tiles [128,512],
    one sigmoid(2*sd2) [128,512]; parity-only tiles take a single
    sigmoid(4000*parT - 2000) [128,512] straight from PSUM. One output DMA
    of [128, 1024] bf16 (absmax tolerance 2e-2; bf16 rounds by <=2^-9 rel).
"""
import os
import numpy as np

W = H = 1024
NCORES = 8
OCT_H = 128          # tile rows
NOCT = 8             # tiles per core
R_KEEP = 2.5         # cull radius: dropping features >2.5px away errs <= e^-12.5
R_WIN = 7.0          # x-window margin around an edge's x-extent
BIG = 4.0e5          # slab test scale: w~ = (2*sqrt(BIG)/L)*(w - L/2)
SQBIG = float(np.sqrt(BIG))
BANDK = 1024.0       # band-mask parity offset (fp16-exact with small ints)
TS_ON_POOL = bool(int(os.environ.get("KERNEL_TS_POOL", "0")))
DUMMY_D2 = 3600.0

LAST_RESULTS = None  # BassKernelResults of the most recent run (for test harness)

_OPS_REGISTERED = {}
_WALRUS_PATCHED = False
MAX_SEM_NUM = int(os.environ.get("KERNEL_MAX_SEM", "64"))


def _patch_walrus_max_sems():
    """Cap walrus' physical semaphore allocation. The codegen epilogue resets
    every allocated semaphore individually (~160ns each per engine), so the
    default full-file allocation costs ~8us of fixed tail per execution."""
    global _WALRUS_PATCHED
    if _WALRUS_PATCHED or MAX_SEM_NUM <= 0:
        return
    import concourse.bass_utils as bu
    orig = bu.run_command

    def wrapped(cmd, *a, **kw):
        if (isinstance(cmd, list) and cmd and "walrus_driver" in str(cmd[0])
                and not any(str(c).startswith("--max-sem-num") for c in cmd)):
            cmd = list(cmd) + [f"--max-sem-num={MAX_SEM_NUM}"]
        return orig(cmd, *a, **kw)

    bu.run_command = wrapped
    _WALRUS_PATCHED = True


# ---------------------------------------------------------------------------
# custom DVE ops (registered into concourse's table at build time)
# ---------------------------------------------------------------------------

def _register_custom_ops():
    global _OPS_REGISTERED
    if _OPS_REGISTERED:
        return _OPS_REGISTERED
    from concourse import dve_ops
    from concourse.dve_spec import (
        Spec, Src0, Src1, C0, C1, C2, One, sq, maxx, minn, scan, lower, AluOp,
    )
    from concourse.dve_uop import DveOpSpec
    from concourse.dve_table_gen import dve_ver_for

    ver = dve_ver_for("TRN2")

    # u_k = C1 + (k+1)*C0  (scan of constant C0 seeded with C1)
    def ref_edge_min(in0, in1, s0, s1, imm2):
        k = np.arange(in0.shape[-1], dtype=np.float32)[None, :]
        u = (s1 + (k + 1.0) * s0).astype(np.float32)
        cand = np.maximum(in0.astype(np.float32) ** 2 - imm2, u * u)
        return np.minimum(in1.astype(np.float32), cand).astype(np.float32)

    def ref_vert_min(in0, in1, s0, s1, imm2):
        k = np.arange(in0.shape[-1], dtype=np.float32)[None, :]
        t = (s0 + (k + 1.0)).astype(np.float32)
        return np.minimum(in0.astype(np.float32), t * t + s1).astype(np.float32)

    specs = {
        # d2 = min(d2, max(w~^2 - BIG, u^2)); in0=w~, in1=d2 (=out), s0=scC,
        # s1=bC(scan-shifted), imm2=BIG
        "POLY_EDGE_MIN": (
            Spec(body=minn(Src1, maxx(sq(Src0) - C2,
                                      sq(scan(AluOp.ADD, C0, init=C1)))),
                 reference=ref_edge_min), True),
        # d2 = min(d2, (scan)^2 + q); in0=d2 (in place), s0=kv(scan-shifted),
        # s1=q
        "POLY_VERT_MIN": (
            Spec(body=minn(Src0, sq(scan(AluOp.ADD, One, init=C0)) + C1),
                 reference=ref_vert_min), False),
    }
    row = max(dve_ops._SUB_OPCODE_FOR_NAME.values(), default=0)
    for name, (spec, rd1) in specs.items():
        if name in dve_ops._SUB_OPCODE_FOR_NAME:
            _OPS_REGISTERED[name] = next(o for o in dve_ops.OPS if o.name == name)
            continue
        row += 1
        assert row < 0x20, "custom-DVE opcode rows exhausted"
        dve_ops._SUB_OPCODE_FOR_NAME[name] = row
        tmp = DveOpSpec(name=name, opcode=row, uops=lower(spec, ver=ver), rd1_en=rd1)
        op = dve_ops.DveOp(name=name, spec=spec, subdim=False,
                           uops_sha={ver: tmp.sha(ver)})
        dve_ops.OPS.append(op)
        _OPS_REGISTERED[name] = op
    return _OPS_REGISTERED


# ---------------------------------------------------------------------------
# host-side geometry prep
# ---------------------------------------------------------------------------

def _seg_hseg_d2(ax, ay, bx, by, x0, x1, y):
    """Squared distance from segment (a,b) to horizontal segment
    [x0,x1] x {y}, vectorized over y (1-D array)."""
    y = np.asarray(y, dtype=np.float64)

    def pt_seg(px, py, sx0, sy0, dx, dy):
        ll = dx * dx + dy * dy
        t = np.clip(((px - sx0) * dx + (py - sy0) * dy) / max(ll, 1e-18), 0, 1)
        return (sx0 + t * dx - px) ** 2 + (sy0 + t * dy - py) ** 2

    abx, aby = bx - ax, by - ay
    # endpoints of edge to hseg: clamp x into [x0,x1]
    d2 = (np.clip(ax, x0, x1) - ax) ** 2 + (y - ay) ** 2
    d2 = np.minimum(d2, (np.clip(bx, x0, x1) - bx) ** 2 + (y - by) ** 2)
    # endpoints of hseg to edge
    d2 = np.minimum(d2, pt_seg(x0, y, ax, ay, abx, aby))
    d2 = np.minimum(d2, pt_seg(x1, y, ax, ay, abx, aby))
    # crossing test: edge crosses the horizontal line at y within [x0,x1]
    if abs(aby) > 1e-18:
        t = (y - ay) / aby
        xc = ax + t * abx
        hit = (t >= 0) & (t <= 1) & (xc >= x0) & (xc <= x1)
        d2 = np.where(hit, 0.0, d2)
    return d2


def _host_prep(polygon):
    poly = np.asarray(polygon, dtype=np.float32)
    E = poly.shape[0]
    a = poly
    b = np.roll(poly, -1, axis=0)
    ab = b - a

    # bbox band (exact f32 replication of the reference)
    x_lo = np.float32(np.floor(poly[:, 0].min()))
    y_lo = np.float32(np.floor(poly[:, 1].min()))
    x_hi = np.float32(np.floor(poly[:, 0].max()) + np.float32(1.0))
    y_hi = np.float32(np.floor(poly[:, 1].max()) + np.float32(1.0))
    thr = np.float32(30.0)
    xband_lo = x_lo - thr
    xband_hi = x_hi + thr
    yband_lo = y_lo - thr
    yband_hi = y_hi + thr

    # ---- signed crossing histogram (exact f32 semantics) ----
    PX = np.arange(W, dtype=np.float32)[None, :]
    a0 = a[:, 0:1]; a1 = a[:, 1:2]; b0 = b[:, 0:1]
    ab0 = ab[:, 0:1]; ab1 = ab[:, 1:2]
    crosses = (a0 <= PX) != (b0 <= PX)                       # [E, W]
    safe_dx = np.where(ab0 == np.float32(0.0), np.float32(1.0), ab0)
    with np.errstate(over='ignore', invalid='ignore'):
        yint = a1 + (PX - a0) * ab1 / safe_dx                # [E, W] f32
    bins = np.where(crosses, np.ceil(yint.astype(np.float64)), np.inf)
    bins = np.where(bins < 0, 0.0, bins)
    bins = np.where(bins > H - 1, np.inf, bins)
    srt = np.sort(bins, axis=0)
    sign = np.where((np.arange(E)[:, None] % 2) == 0, 1.0, -1.0)
    hist = np.zeros((H, W), dtype=np.float64)
    valid = np.isfinite(srt)
    kk = srt[valid].astype(np.int64)
    jj = np.broadcast_to(np.arange(W)[None, :], (E, W))[valid]
    np.add.at(hist, (kk, jj), np.broadcast_to(sign, (E, W))[valid])
    csum = np.cumsum(hist, axis=0)

    r_lo = int(np.ceil(float(yband_lo)))
    r_hi = int(np.floor(float(yband_hi)))
    xmask = ~((np.arange(W) >= float(xband_lo)) & (np.arange(W) <= float(xband_hi)))

    # ---- per-(row, tile) packed candidate lists (f64 geometry) ----
    A = a.astype(np.float64); B = b.astype(np.float64); AB = B - A
    L2 = AB[:, 0] ** 2 + AB[:, 1] ** 2
    Lc = np.sqrt(np.maximum(L2, 1e-12))
    good = L2 > 1e-9

    # row_edges[s][o][j] = list of edge ids (sorted by x-center);
    # row_verts[s][o][j] = list of vertex (edge) ids
    row_edges = [[[[] for _ in range(128)] for _ in range(NOCT)] for _ in range(8)]
    row_verts = [[[[] for _ in range(128)] for _ in range(NOCT)] for _ in range(8)]
    for e in range(E):
        ax, ay = A[e]; bx, by = B[e]
        if good[e]:
            ylo = max(0, int(np.floor(min(ay, by) - R_KEEP)))
            yhi = min(H - 1, int(np.ceil(max(ay, by) + R_KEEP)))
            for s in range(8):
                x0, x1 = s * 128, s * 128 + 127
                if max(ax, bx) < x0 - R_KEEP or min(ax, bx) > x1 + R_KEEP:
                    continue
                ys = np.arange(ylo, yhi + 1)
                d2r = _seg_hseg_d2(ax, ay, bx, by, x0, x1, ys)
                for y, dd in zip(ys, d2r):
                    if dd <= R_KEEP * R_KEEP:
                        row_edges[s][(y // OCT_H)][y % OCT_H].append(e)
        # vertex a of edge e
        s0v = max(0, int(np.floor(ax - R_KEEP)) // 128)
        s1v = min(7, int(np.ceil(ax + R_KEEP)) // 128)
        ylo = max(0, int(np.floor(ay - R_KEEP)))
        yhi = min(H - 1, int(np.ceil(ay + R_KEEP)))
        for s in range(s0v, s1v + 1):
            for y in range(ylo, yhi + 1):
                row_verts[s][y // OCT_H][y % OCT_H].append(e)
    # sort each row's edge list by x-center so slots cluster in x
    xc = (A[:, 0] + B[:, 0]) / 2
    for s in range(8):
        for o in range(NOCT):
            for j in range(128):
                row_edges[s][o][j].sort(key=lambda e: xc[e])
                row_verts[s][o][j].sort(key=lambda e: A[e, 0])

    nS = np.zeros((8, NOCT), dtype=int)
    nV = np.zeros((8, NOCT), dtype=int)
    for s in range(8):
        for o in range(NOCT):
            nS[s, o] = max(len(r) for r in row_edges[s][o])
            nV[s, o] = max(len(r) for r in row_verts[s][o])
            if nV[s, o] > 0 and nS[s, o] == 0:
                nS[s, o] = 1

    # per-(tile, slot) x-windows (local cols, padded to mult of 4)
    def slot_windows(s, o):
        wins_e, wins_v = [], []
        for si in range(nS[s, o]):
            lo, hi = 128, 0
            for j in range(128):
                lst = row_edges[s][o][j]
                if si < len(lst):
                    e = lst[si]
                    lo = min(lo, min(A[e, 0], B[e, 0]) - R_WIN - s * 128)
                    hi = max(hi, max(A[e, 0], B[e, 0]) + R_WIN - s * 128)
            lo = int(max(0, np.floor(lo))); hi = int(min(127, np.ceil(hi)))
            wins_e.append((lo, hi + 1) if lo <= hi else (0, 4))
        for vi in range(nV[s, o]):
            lo, hi = 128, 0
            for j in range(128):
                lst = row_verts[s][o][j]
                if vi < len(lst):
                    e = lst[vi]
                    lo = min(lo, A[e, 0] - R_WIN - s * 128)
                    hi = max(hi, A[e, 0] + R_WIN - s * 128)
            lo = int(max(0, np.floor(lo))); hi = int(min(127, np.ceil(hi)))
            wins_v.append((lo, hi + 1) if lo <= hi else (0, 4))
        return wins_e, wins_v

    tile_wins = {(s, o): slot_windows(s, o) for s in range(8) for o in range(NOCT)}

    # ---- tile -> (core, phase) assignment (balance padded window costs) ----
    octs = [(s, o) for s in range(8) for o in range(NOCT)]
    CE_FIX, CV_FIX = 140.0, 140.0   # per-op fixed ns
    def tile_cost(so):
        we, wv = tile_wins[so]
        c = sum(2 * (hi - lo) * 2.1 + 3 * CE_FIX for lo, hi in we)
        c += sum((hi - lo) * 2.1 + CV_FIX for lo, hi in wv)
        return c
    cost = {so: tile_cost(so) for so in octs}

    order = sorted(octs, key=lambda so: -cost[so])
    core_load = [0.0] * NCORES
    assign = [[] for _ in range(NCORES)]
    for so in order:
        cands = [c for c in range(NCORES) if len(assign[c]) < NOCT]
        c = min(cands, key=lambda c: core_load[c])
        assign[c].append(so)
        core_load[c] += cost[so]

    def padded_cost(asg):
        ranked = [sorted(aa, key=lambda so: -cost[so]) for aa in asg]
        tot = 0.0
        for k in range(NOCT):
            tiles = [r[k] for r in ranked]
            smax = max(nS[t] for t in tiles)
            vmax = max(nV[t] for t in tiles)
            for si in range(smax):
                lo = min(tile_wins[t][0][si][0] for t in tiles
                         if si < len(tile_wins[t][0])) if smax else 0
                hi = max(tile_wins[t][0][si][1] for t in tiles
                         if si < len(tile_wins[t][0])) if smax else 0
                tot += 2 * (hi - lo) * 2.1 + 3 * CE_FIX
            for vi in range(vmax):
                lo = min(tile_wins[t][1][vi][0] for t in tiles
                         if vi < len(tile_wins[t][1])) if vmax else 0
                hi = max(tile_wins[t][1][vi][1] for t in tiles
                         if vi < len(tile_wins[t][1])) if vmax else 0
                tot += (hi - lo) * 2.1 + CV_FIX
        return tot

    best = padded_cost(assign)
    rng = np.random.default_rng(0)
    for _ in range(4000):
        c1, c2 = rng.integers(0, NCORES, 2)
        if c1 == c2:
            continue
        i1, i2 = rng.integers(0, NOCT, 2)
        assign[c1][i1], assign[c2][i2] = assign[c2][i2], assign[c1][i1]
        newc = padded_cost(assign)
        if newc <= best:
            best = newc
        else:
            assign[c1][i1], assign[c2][i2] = assign[c2][i2], assign[c1][i1]
    core_octs = [sorted(aa, key=lambda so: -cost[so]) for aa in assign]

    # padded per-phase slot counts + windows
    S = []; V = []; EWIN = []; VWIN = []
    for k in range(NOCT):
        tiles = [core_octs[c][k] for c in range(NCORES)]
        smax = int(max(nS[t] for t in tiles))
        vmax = int(max(nV[t] for t in tiles))
        ew = []
        for si in range(smax):
            lo = min((tile_wins[t][0][si][0] for t in tiles
                      if si < len(tile_wins[t][0])), default=0)
            hi = max((tile_wins[t][0][si][1] for t in tiles
                      if si < len(tile_wins[t][0])), default=4)
            ext = hi - lo
            ext = min(128, (ext + 3) // 4 * 4)
            lo = min(lo, 128 - ext)
            ew.append((lo, ext))
        vw = []
        for vi in range(vmax):
            lo = min((tile_wins[t][1][vi][0] for t in tiles
                      if vi < len(tile_wins[t][1])), default=0)
            hi = max((tile_wins[t][1][vi][1] for t in tiles
                      if vi < len(tile_wins[t][1])), default=4)
            ext = hi - lo
            ext = min(128, (ext + 3) // 4 * 4)
            lo = min(lo, 128 - ext)
            vw.append((lo, ext))
        S.append(smax); V.append(vmax); EWIN.append(ew); VWIN.append(vw)

    # edge phases = prefix with S[k] > 0 (cost-sorted so this is a prefix)
    NE = sum(1 for k in range(NOCT) if S[k] > 0)
    assert NE <= 4, f"more than 4 edge phases per core ({NE}); layout assumes <=4"
    NE_PAD = 4  # blocks 0..3 are edge-final tiles, 4..7 parity tiles

    # ---- layout of the per-phase PE matmul producing w~ and vertex fields --
    # phase p: lhsT rows = [bt,st]*S[p] then [c0,c1,c2]*V[p]; rhs (aux,
    # identical across cores) is block-diagonal with blocks [1; xl] per edge
    # slot and [1; xtl; xtl^2] per vertex slot over each slot's x-window.
    KROW = [2 * S[k] + 3 * V[k] for k in range(NE)]
    assert all(kr <= 32 for kr in KROW), KROW
    # matmul lhsT base partition must be 0/32/64: keep all phases in rows
    # 0..31 and put the phase along the free axis instead
    KOFF = [0] * (NE + 1)
    KTOT = 32
    ECOL = []   # per phase: list of col offsets per edge slot (within phase)
    VCOL = []
    CW = []     # per-phase col width
    for k in range(NE):
        ec = []; off = 0
        for si in range(S[k]):
            ec.append(off); off += EWIN[k][si][1]
        vc = []
        for vi in range(V[k]):
            vc.append(off); off += VWIN[k][vi][1]
        ECOL.append(ec); VCOL.append(vc); CW.append(off)
    COFF = [0]
    for k in range(NE):
        COFF.append(COFF[-1] + CW[k])
    CTOT = max(COFF[-1], 4)

    aux = np.zeros((KTOT, CTOT), dtype=np.float32)
    for k in range(NE):
        for si in range(S[k]):
            xoff, ext = EWIN[k][si]
            c0 = COFF[k] + ECOL[k][si]
            r = 2 * si
            aux[r + 0, c0:c0 + ext] = 1.0
            aux[r + 1, c0:c0 + ext] = np.arange(xoff, xoff + ext)
        for vi in range(V[k]):
            xoff, ext = VWIN[k][vi]
            c0 = COFF[k] + VCOL[k][vi]
            r = 2 * S[k] + 3 * vi
            xt = np.arange(xoff, xoff + ext, dtype=np.float64) - 64.0
            aux[r + 0, c0:c0 + ext] = 1.0
            aux[r + 1, c0:c0 + ext] = xt
            aux[r + 2, c0:c0 + ext] = xt * xt

    # ---- per-core input tensors ----
    # coef: per edge slot (scC, bCs) + final parity-bias column; coefT: the
    # transposed per-row matmul coefficients [KTOT, 128]
    ncol = sum(2 * S[k] for k in range(NE)) + 1
    in_maps = []
    for c in range(NCORES):
        coef = np.zeros((128, ncol), dtype=np.float32)
        coef[:, ncol - 1] = -2000.0
        # coefT: [32 rows, NE*128]: phase k occupies free cols k*128..
        coefT = np.zeros((KTOT, max(NE, 1) * 128), dtype=np.float32)
        histc = np.zeros((NOCT, OCT_H, 128), dtype=np.float64)
        col = 0
        for k in range(NOCT):
            s, o = core_octs[c][k]
            i0 = o * OCT_H
            # --- fp16 histogram block with band + base + xmask folded in ---
            hloc = np.ascontiguousarray(hist[i0:i0 + OCT_H, s * 128:(s + 1) * 128])
            if i0 > 0:
                base = np.mod(csum[i0 - 1, s * 128:(s + 1) * 128], 2.0)
                hloc[0, :] += base
            xm = xmask[s * 128:(s + 1) * 128]
            rl = r_lo - i0          # first in-band local row
            rh1 = r_hi + 1 - i0     # first out-of-band local row above
            ymask0 = np.zeros(128)  # row-0 offset for y-band
            if rl > 0:
                ymask0 -= BANDK
            if rh1 <= 0:
                ymask0 -= BANDK
            # out-of-x-band columns: flat -BANDK, no y-steps
            hloc[0, :] += np.where(xm, -BANDK, ymask0)
            if 0 < rl <= OCT_H - 1:
                hloc[rl, :] += np.where(xm, 0.0, BANDK)
            if 0 < rh1 <= OCT_H - 1:
                hloc[rh1, :] += np.where(xm, 0.0, -BANDK)
            histc[k] = hloc

            if k >= NE:
                continue
            # --- packed per-row slot coefficients ---
            eg = row_edges[s][o]
            vt = row_verts[s][o]
            for si in range(S[k]):
                xoff, ext = EWIN[k][si]
                r = 2 * si
                sc_c = np.zeros(128, dtype=np.float64)
                bcs_c = np.full(128, 60.0, dtype=np.float64)   # dummy: u=60
                for j in range(128):
                    lst = eg[j]
                    if si < len(lst):
                        e = lst[si]
                        y = i0 + j
                        L = Lc[e]
                        sig = 2.0 * SQBIG / L
                        # w~(xl) = bt + st*xl
                        coefT[r + 1, k * 128 + j] = sig * AB[e, 0] / L
                        coefT[r + 0, k * 128 + j] = sig * (
                            (s * 128 - A[e, 0]) * AB[e, 0] / L
                            + (y - A[e, 1]) * AB[e, 1] / L - L / 2.0)
                        scC = AB[e, 1] / L
                        bC = ((s * 128 - A[e, 0]) * AB[e, 1] / L
                              - (y - A[e, 1]) * AB[e, 0] / L)
                        sc_c[j] = scC
                        # scan u_k = init + (k+1)*scC; want scC*(xoff+k) + bC
                        bcs_c[j] = bC + scC * (xoff - 1)
                coef[:, col + 0] = sc_c
                coef[:, col + 1] = bcs_c
                col += 2
            for vi in range(V[k]):
                xoff, ext = VWIN[k][vi]
                r = 2 * S[k] + 3 * vi
                for j in range(128):
                    lst = vt[j]
                    if vi < len(lst):
                        e = lst[vi]
                        y = i0 + j
                        # v = (xtl - d)^2 + q, xtl = xl-64, d = (Ax-s*128)-64
                        d = (A[e, 0] - s * 128) - 64.0
                        q = (y - A[e, 1]) ** 2
                        coefT[r + 0, k * 128 + j] = d * d + q
                        coefT[r + 1, k * 128 + j] = -2.0 * d
                        coefT[r + 2, k * 128 + j] = 1.0
                    else:
                        coefT[r + 0, k * 128 + j] = DUMMY_D2
        hist16 = histc.astype(np.float16)
        assert np.all(hist16.astype(np.float64) == histc), "hist not fp16-exact"
        in_maps.append({
            "coef": coef,
            "coefT": coefT,
            "aux": aux,
            "hist": np.ascontiguousarray(
                histc.transpose(1, 0, 2).reshape(OCT_H, NOCT * 128)
            ).astype(np.float16),
        })

    meta = dict(S=S, V=V, EWIN=EWIN, VWIN=VWIN, NE=NE, ncol=ncol,
                KROW=KROW, KOFF=KOFF, KTOT=max(KTOT, 1), ECOL=ECOL, VCOL=VCOL,
                CW=CW, COFF=COFF, CTOT=CTOT, core_octs=core_octs)
    return in_maps, meta


# ---------------------------------------------------------------------------
# numpy simulator of the device program (host-side debugging)
# ---------------------------------------------------------------------------

def _simulate(in_maps, meta):
    S, V = meta["S"], meta["V"]
    EWIN, VWIN = meta["EWIN"], meta["VWIN"]
    NE = meta["NE"]
    KOFF, ECOL, VCOL, COFF = meta["KOFF"], meta["ECOL"], meta["VCOL"], meta["COFF"]
    core_octs = meta["core_octs"]
    outs = []
    U = (np.arange(128)[:, None] <= np.arange(128)[None, :])  # U[y,i]
    for c in range(NCORES):
        coef = in_maps[c]["coef"].astype(np.float64)
        coefT = in_maps[c]["coefT"].astype(np.float64)
        aux = in_maps[c]["aux"].astype(np.float64)
        hall = in_maps[c]["hist"].astype(np.float64)  # [y, k*128+x]
        out = np.zeros((NOCT, 128, 128), dtype=np.float64)  # [k, i, x]
        parT = np.einsum('yc,yi->ic', hall, U)  # [i, NOCT*128]
        col = 0
        for k in range(NOCT):
            par = parT[:, k * 128:(k + 1) * 128]  # [i, x]
            if k >= NE:
                out[k] = 1.0 / (1.0 + np.exp(-np.clip(4000.0 * par - 2000.0,
                                                      -700, 700)))
                continue
            # PE matmul producing the phase's w~ / vertex fields
            wq = coefT[:, k * 128:(k + 1) * 128].T @ aux[:,
                                                         COFF[k]:COFF[k + 1]]
            d2 = np.full((128, 128), DUMMY_D2)
            for si in range(S[k]):
                xoff, ext = EWIN[k][si]
                sc = coef[:, col + 0:col + 1]; bcs = coef[:, col + 1:col + 2]
                wt = wq[:, ECOL[k][si]:ECOL[k][si] + ext]
                kk = np.arange(ext, dtype=np.float64)[None, :]
                u = bcs + (kk + 1.0) * sc
                cand = np.maximum(wt * wt - BIG, u * u)
                d2[:, xoff:xoff + ext] = np.float16(
                    np.minimum(d2[:, xoff:xoff + ext], cand))
                col += 2
            for vi in range(V[k]):
                xoff, ext = VWIN[k][vi]
                v = wq[:, VCOL[k][vi]:VCOL[k][vi] + ext]
                d2[:, xoff:xoff + ext] = np.float16(
                    np.minimum(d2[:, xoff:xoff + ext], v))
            sd2 = (par - 0.5) * d2
            out[k] = 1.0 / (1.0 + np.exp(-np.clip(2.0 * sd2, -700, 700)))
        outs.append(out.astype(np.float32))
    # assemble
    full = np.zeros((H, W), dtype=np.float32)
    for c in range(NCORES):
        for k in range(NOCT):
            s, o = core_octs[c][k]
            full[o * 128:(o + 1) * 128, s * 128:(s + 1) * 128] = outs[c][k]
    return full


# ---------------------------------------------------------------------------
# device program
# ---------------------------------------------------------------------------

def _build_program(meta):
    import concourse.bacc as bacc
    import concourse.mybir as mybir
    from concourse.tile import TileContext

    ops = _register_custom_ops()
    EDGE_MIN = ops["POLY_EDGE_MIN"]
    VERT_MIN = ops["POLY_VERT_MIN"]

    F32 = mybir.dt.float32
    F16 = mybir.dt.float16
    BF16 = mybir.dt.bfloat16
    I32 = mybir.dt.int32
    AF = mybir.ActivationFunctionType
    OP = mybir.AluOpType

    S, V = meta["S"], meta["V"]
    EWIN, VWIN = meta["EWIN"], meta["VWIN"]
    NE, ncol = meta["NE"], meta["ncol"]
    KOFF, KTOT = meta["KOFF"], meta["KTOT"]
    ECOL, VCOL = meta["ECOL"], meta["VCOL"]
    CW, COFF, CTOT = meta["CW"], meta["COFF"], meta["CTOT"]

    nc = bacc.Bacc()
    coef_in = nc.declare_dram_parameter("coef", [128, ncol], F32, isOutput=False)
    coefT_in = nc.declare_dram_parameter("coefT", [KTOT, max(NE, 1) * 128], F32,
                                         isOutput=False)
    aux_in = nc.declare_dram_parameter("aux", [KTOT, CTOT], F32, isOutput=False)
    hist_in = nc.declare_dram_parameter("hist", [OCT_H, NOCT * 128], F16,
                                        isOutput=False)
    out_dram = nc.declare_dram_parameter("out", [128, NOCT * 128], BF16,
                                         isOutput=True)

    with TileContext(nc) as tc:
        with tc.tile_pool(name="const", bufs=1) as cpool, \
             tc.tile_pool(name="work", bufs=4) as wpool, \
             tc.tile_pool(name="ps", bufs=1, space="PSUM") as psum, \
             tc.tile_pool(name="psw", bufs=2, space="PSUM") as psumw:

            coef = cpool.tile([128, ncol], F32)
            nc.sync.dma_start(out=coef[:], in_=coef_in[:])
            coefT = cpool.tile([KTOT, max(NE, 1) * 128], F32)
            nc.sync.dma_start(out=coefT[:], in_=coefT_in[:])
            aux = cpool.tile([KTOT, CTOT], F32)
            nc.sync.dma_start(out=aux[:], in_=aux_in[:])
            hall = cpool.tile([128, NOCT * 128], F16)
            nc.sync.dma_start(out=hall[:], in_=hist_in[:])

            # warmup: trigger ACT table load while DMAs are in flight
            warm = cpool.tile([128, 1], F32)
            nc.vector.memset(warm[:], 0.0)
            nc.scalar.activation(warm[:], warm[:], AF.Sigmoid, bias=0.0, scale=1.0)

            # ubf fp16 triangular U[y,i] = (i >= y)
            ui = cpool.tile([128, 128], I32)
            nc.gpsimd.iota(ui[:], pattern=[[1, 128]], base=0, channel_multiplier=-1)
            ubf = cpool.tile([128, 128], F16)
            nc.vector.tensor_scalar(out=ubf[:], in0=ui[:], scalar1=0, scalar2=None,
                                    op0=OP.is_ge)

            # one matmul pair: parT[i, k*128+x] for all 8 tiles
            parT = psum.tile([128, NOCT * 128], F32)
            nc.tensor.matmul(parT[:, 0:512], lhsT=ubf[:], rhs=hall[:, 0:512],
                             start=True, stop=True)
            nc.tensor.matmul(parT[:, 512:1024], lhsT=ubf[:], rhs=hall[:, 512:1024],
                             start=True, stop=True)

            # parity-only tiles: val = sigmoid(4000*parT - 2000); emitted
            # early so the ACT + its DMA overlap the DVE distance block
            val = wpool.tile([128, NOCT * 128], BF16, tag="val")
            nc.scalar.activation(val[:, 512:1024], parT[:, 512:1024], AF.Sigmoid,
                                 bias=coef[:, ncol - 1:ncol], scale=4000.0)
            nc.sync.dma_start(out=out_dram[:, 512:1024], in_=val[:, 512:1024])

            # d2 for the 4 edge tiles
            d2q = wpool.tile([128, 4 * 128], F16, tag="d2q")
            nc.vector.memset(d2q[:], DUMMY_D2)

            sd2q = wpool.tile([128, 4 * 128], F32, tag="sd2q")
            col = 0
            for k in range(NE):
                d2 = d2q[:, k * 128:(k + 1) * 128]
                # one PE matmul per phase: w~ fields + vertex candidate planes
                wq = psumw.tile([128, max(CW[k], 4)], F32, tag="wq")
                for c0 in range(0, CW[k], 512):
                    c1 = min(CW[k], c0 + 512)
                    nc.tensor.matmul(
                        wq[:, c0:c1],
                        lhsT=coefT[:, k * 128:(k + 1) * 128],
                        rhs=aux[:, COFF[k] + c0:COFF[k] + c1],
                        start=True, stop=True)
                for si in range(S[k]):
                    xoff, ext = EWIN[k][si]
                    wt = wq[:, ECOL[k][si]:ECOL[k][si] + ext]
                    nc.vector._custom_dve(
                        EDGE_MIN, out=d2[:, xoff:xoff + ext], in0=wt,
                        in1=d2[:, xoff:xoff + ext],
                        s0=coef[:, col + 0:col + 1],
                        s1=coef[:, col + 1:col + 2], imm2=BIG)
                    col += 2
                for vi in range(V[k]):
                    xoff, ext = VWIN[k][vi]
                    nc.vector.tensor_tensor(
                        out=d2[:, xoff:xoff + ext], in0=d2[:, xoff:xoff + ext],
                        in1=wq[:, VCOL[k][vi]:VCOL[k][vi] + ext], op=OP.min)
                # finals per pair of phases, pipelined with the next phases
                if k % 2 == 1 or k == NE - 1:
                    lo = (k // 2) * 256
                    hi = lo + 256 if k % 2 == 1 else lo + 128
                    nc.vector.scalar_tensor_tensor(
                        out=sd2q[:, lo:hi], in0=parT[:, lo:hi], scalar=0.5,
                        in1=d2q[:, lo:hi], op0=OP.subtract, op1=OP.mult)
                    nc.scalar.activation(val[:, lo:hi], sd2q[:, lo:hi],
                                         AF.Sigmoid, bias=0.0, scale=2.0)
                    nc.sync.dma_start(out=out_dram[:, lo:hi], in_=val[:, lo:hi])
            if NE < 4:
                # untouched edge blocks (parity-only): d2 = DUMMY everywhere
                lo = ((NE + 1) // 2) * 256 if NE % 2 == 0 else NE * 128
                nc.vector.scalar_tensor_tensor(
                    out=sd2q[:, lo:512], in0=parT[:, lo:512], scalar=0.5,
                    in1=d2q[:, lo:512], op0=OP.subtract, op1=OP.mult)
                nc.scalar.activation(val[:, lo:512], sd2q[:, lo:512],
                                     AF.Sigmoid, bias=0.0, scale=2.0)
                nc.sync.dma_start(out=out_dram[:, lo:512], in_=val[:, lo:512])

    nc.finalize()
    return nc


# ---------------------------------------------------------------------------
# entry point
# ---------------------------------------------------------------------------

def kernel(polygon):
    global LAST_RESULTS
    from concourse.bass_utils import run_bass_kernel_spmd

    _patch_walrus_max_sems()
    in_maps, meta = _host_prep(polygon)
    nc = _build_program(meta)
    trace = bool(int(os.environ.get("KERNEL_TRACE", "0")))
    res = run_bass_kernel_spmd(nc, in_maps, list(range(NCORES)), trace=trace)
    LAST_RESULTS = res

    core_octs = meta["core_octs"]
    full = np.zeros((H, W), dtype=np.float32)
    for c in range(NCORES):
        o = np.asarray(res.results[c]["out"]).astype(np.float32)  # [128, 8*128]
        for k in range(NOCT):
            s, oq = core_octs[c][k]
            full[oq * 128:(oq + 1) * 128, s * 128:(s + 1) * 128] = \
                o[:, k * 128:(k + 1) * 128]
    return np.ascontiguousarray(full)
